# revision 1
# baseline (speedup 1.0000x reference)
"""ClosestPointLoss kernel for 8 trn2 NeuronCores.

mean_i min_j ||outputs_i - targets_j||^2 over outputs [131072,3], targets [16384,3].

Strategy (per sharding hint): shard `outputs` rows 8 ways, replicate `targets`.
Per core: dist^2(i,j) = ||a_i||^2 + (||t_j||^2 - 2 a_i.t_j). The parenthesized
term is a K=6 matmul of augmented vectors w=[1,1,1,a], r=[t^2,-2t]. For exact
fp32 precision at bf16 matmul speed, each fp32 value is split into 3 bf16
levels (hi/lo/l2) and the 6 significant cross products are stacked into a
single K=36 bf16 matmul (block-diagonal stacking along the contraction dim).
Two matmuls run concurrently in PE row groups (tile_position 0/64).
Row-wise min over the 16384 targets: ScalarE copies half the PSUM banks to
SBUF; a custom DVE op (min(in0,in1) elementwise + min-reduce, seeded by the
running min) consumes a PSUM stream and an SBUF stream at 2 values/cycle.
Per-core output: [128,144] = 128 cols of row-mins (one per 128-point tile)
+ 8 cols of sum(a^2) partials; host does the final fp64 sum / 131072.
"""
import sys

sys.path.insert(0, "/opt/trn_rl_repo")

import numpy as np
from contextlib import ExitStack

N_CORES = 8
NPTS = 131072
NT = 16384          # targets (also points per core)
PP = NPTS // N_CORES  # 16384 points per core
M = 128             # points per weight tile
NMT = PP // M       # 128 m-tiles per core
NCH = 512           # matmul moving free dim (1 psum bank)
UNIT = 2048         # targets per pipeline unit (2 direct + 2 copied banks)
NU = NT // UNIT     # 8 units per m-tile

_compiled = None


def _register_min_min_reduce():
    from concourse import dve_ops
    from concourse.dve_ops import DveOp, OPS, _SUB_OPCODE_FOR_NAME, _CUSTOM_DVE_ROW_BASE
    from concourse.dve_spec import Spec, Src0, Src1, C0, minn

    if "MIN_MIN_REDUCE" in _SUB_OPCODE_FOR_NAME:
        return dve_ops.MIN_MIN_REDUCE
    def _mmr_ref(in0, in1, c0, c1, c2):
        body = np.minimum(in0.astype(np.float32), in1.astype(np.float32))
        acc = np.minimum(np.asarray(c0, np.float32), body.min(axis=-1, keepdims=True))
        return body, acc

    op = DveOp(
        "MIN_MIN_REDUCE",
        Spec(
            body=minn(Src0, Src1),
            accum=minn,
            accum_init=C0,
            reference=_mmr_ref,
        ),
        subdim=False,
        uops_sha={},
    )
    from concourse.dve_ops import DveOpSpec, lower, has_src1

    for ver in ("v3", "v4"):
        spec = DveOpSpec(name=op.name, opcode=0, uops=lower(op.spec, ver=ver),
                         rd1_en=has_src1(op.spec))
        op.uops_sha[ver] = spec.sha(ver)
    OPS.append(op)
    _SUB_OPCODE_FOR_NAME[op.name] = _CUSTOM_DVE_ROW_BASE + len(OPS) - 1
    dve_ops.CUSTOM_DVE_SPECS[op.name] = op.spec
    dve_ops.MIN_MIN_REDUCE = op
    return op


def _build():
    import concourse.bacc as bacc
    import concourse.tile as tile
    from concourse import mybir

    MMR = _register_min_min_reduce()
    AL = mybir.AluOpType
    f32 = mybir.dt.float32
    bf16 = mybir.dt.bfloat16

    nc = bacc.Bacc("TRN2", target_bir_lowering=False, debug=False)
    outT = nc.dram_tensor("outT", [3, PP], f32, kind="ExternalInput")   # shard coords, transposed
    tT = nc.dram_tensor("tT", [3, NT], f32, kind="ExternalInput")       # targets, transposed
    out = nc.dram_tensor("out", [128, 144], f32, kind="ExternalOutput")
    w36d = nc.dram_tensor("w36d", [36, PP], bf16, kind="Internal")      # DRAM-assembled W stack
    r36d = nc.dram_tensor("r36d", [36, NT], bf16, kind="Internal")      # DRAM-assembled R stack

    # W blocks (rows 6b..6b+5): level of the ones-channel (rows +0..2) and
    # a-channel (rows +3..5) for block b. R blocks: t^2-channel / -2t-channel.
    W_LEVELS = ["hi", "hi", "lo", "hi", "l2", "lo"]
    R_LEVELS = ["hi", "lo", "hi", "l2", "hi", "lo"]

    with tile.TileContext(nc) as tc:
        with ExitStack() as ctx:
            singles = ctx.enter_context(tc.tile_pool(name="singles", bufs=1))
            W36 = singles.tile([128, PP], bf16)
            R36 = singles.tile([128, NT], bf16)
            out_sb = singles.tile([128, 144], f32)

            # ---------- prep ----------
            # All elementwise prep math runs in a [PR, FC] reshape of the
            # flat [3, N] data (same flat element order, 128x the lanes).
            import concourse.bass as bass

            def flat_rows(dram_ap, r0, nrows, ncols):
                """[nrows, ncols] rows of a DRAM tensor viewed as [PR, FC]."""
                flat = 3 * ncols  # unused; kept for clarity
                v = dram_ap[r0:r0 + nrows, :]
                c = ncols // 512
                return v.rearrange("a (c d) -> (a c) d", c=c, d=512)

            with tc.tile_pool(name="prep_a", bufs=1) as prep_a, \
                 tc.tile_pool(name="prep_lvl", bufs=2) as prep_lvl:
                PRW = 3 * PP // 512   # partitions of [*, 512] view of [3, PP]
                assert PRW <= 128
                a_f32 = prep_a.tile([PRW, 512], f32)
                nc.sync.dma_start(out=a_f32, in_=flat_rows(outT.ap(), 0, 3, PP))

                # ones / zeros rows of the ones-channel (rows 6b..6b+2)
                const_t = prep_a.tile([PRW, 512], bf16, name="const_t", tag="const_t")
                nc.vector.memset(const_t[:, :], 1.0)
                const_z = prep_a.tile([PRW, 512], bf16, name="const_z", tag="const_z")
                nc.vector.memset(const_z[:, :], 0.0)
                for b, lv in enumerate(W_LEVELS):
                    src = const_t if lv == "hi" else const_z
                    nc.sync.dma_start(out=flat_rows(w36d.ap(), 6 * b, 3, PP), in_=src[:, :])

                # sum(a^2) -> out_sb[:, 128] (per-lane partials; host sums)
                nc.vector.memset(out_sb[:, :], 0.0)
                sq = prep_lvl.tile([PRW, 512], f32, name="sqa", tag="sqa")
                nc.vector.tensor_tensor(out=sq, in0=a_f32, in1=a_f32, op=AL.mult)
                nc.vector.tensor_reduce(out=out_sb[0:PRW, 128:129], in_=sq,
                                        axis=mybir.AxisListType.X, op=AL.add)

                # 3-level split of a into w36d a-channel rows (6b+3..6b+5)
                for lv in ("hi", "lo", "l2"):
                    lvt = prep_lvl.tile([PRW, 512], bf16, name="lvw", tag="lvw")
                    nc.scalar.copy(lvt[:, :], a_f32[:, :])  # cast f32->bf16
                    for b, blv in enumerate(W_LEVELS):
                        if blv == lv:
                            nc.sync.dma_start(out=flat_rows(w36d.ap(), 6 * b + 3, 3, PP), in_=lvt[:, :])
                    if lv != "l2":
                        nc.vector.tensor_tensor(out=a_f32[:, :], in0=a_f32[:, :], in1=lvt[:, :],
                                                op=AL.subtract)
                nc.sync.dma_start(out=W36[0:36, :], in_=w36d.ap())
                nc.sync.dma_start(out=W36[64:100, :], in_=w36d.ap())

            # ---------- prep: R side (t^2 and -2t) ----------
            with tc.tile_pool(name="prep_t", bufs=1) as prep_t, \
                 tc.tile_pool(name="prep_lvl2", bufs=2) as prep_lvl2:
                PRT = 3 * NT // 512
                assert PRT <= 128
                t_f32 = prep_t.tile([PRT, 512], f32, name="tf", tag="tf")
                t2_f32 = prep_t.tile([PRT, 512], f32, name="t2f", tag="t2f")
                nc.sync.dma_start(out=t_f32, in_=flat_rows(tT.ap(), 0, 3, NT))
                nc.vector.tensor_tensor(out=t2_f32, in0=t_f32, in1=t_f32, op=AL.mult)
                nc.vector.tensor_scalar_mul(t_f32, t_f32, -2.0)
                for data, rowoff in ((t2_f32, 0), (t_f32, 3)):
                    for lv in ("hi", "lo", "l2"):
                        lvt = prep_lvl2.tile([PRT, 512], bf16, name="lvr", tag="lvr")
                        nc.scalar.copy(lvt[:, :], data[:, :])
                        for b, blv in enumerate(R_LEVELS):
                            if blv == lv:
                                nc.sync.dma_start(out=flat_rows(r36d.ap(), 6 * b + rowoff, 3, NT), in_=lvt[:, :])
                        if lv != "l2":
                            nc.vector.tensor_tensor(out=data[:, :], in0=data[:, :], in1=lvt[:, :],
                                                    op=AL.subtract)
                nc.sync.dma_start(out=R36[0:36, :], in_=r36d.ap())
                nc.sync.dma_start(out=R36[64:100, :], in_=r36d.ap())

            # ---------- main loop ----------
            # unit = 4096 target-cols: 4 "copied" MMs (2 pc tiles -> ACT -> SBUF)
            # + 4 "direct" MMs (pd, 4 banks); one mmr2048 consumes 4096 values.
            pd_pool = ctx.enter_context(tc.tile_pool(name="pd", bufs=2, space="PSUM"))
            pc_pool = ctx.enter_context(tc.tile_pool(name="pc", bufs=2, space="PSUM"))
            cp_pool = ctx.enter_context(tc.tile_pool(name="cp", bufs=3))
            acc_pool = ctx.enter_context(tc.tile_pool(name="accp", bufs=4))
            dump_pool = ctx.enter_context(tc.tile_pool(name="dump", bufs=2))

            def mm_pair(dst, ms, col0):
                nc.tensor.matmul(dst[:, 0:512], W36[0:36, ms], R36[0:36, col0:col0 + 512],
                                 start=True, stop=True, tile_position=(0, 0))
                nc.tensor.matmul(dst[:, 512:1024], W36[64:100, ms],
                                 R36[64:100, col0 + 512:col0 + 1024],
                                 start=True, stop=True, tile_position=(64, 0))

            for m in range(NMT):
                ms = slice(m * M, (m + 1) * M)
                chain = 3.0e38
                for u in range(NU):
                    b0 = u * UNIT
                    pc = pc_pool.tile([128, 1024], f32, name="pct", tag="pct")
                    mm_pair(pc, ms, b0)
                    cpt = cp_pool.tile([128, 1024], f32, name="cpt", tag="cpt")
                    nc.scalar.copy(cpt[:, :], pc[:, :])
                    pd = pd_pool.tile([128, 1024], f32, name="pdt", tag="pdt")
                    mm_pair(pd, ms, b0 + 1024)
                    dump = dump_pool.tile([128, 1], f32, name="dmp", tag="dmp")
                    acc_dst = out_sb[:, m:m + 1] if u == NU - 1 else \
                        acc_pool.tile([128, 1], f32, name="acct", tag="acct")
                    nc.vector._custom_dve(MMR, out=dump.broadcast_to(pd.shape),
                                          in0=pd[:, :], in1=cpt[:, :], s0=chain,
                                          accum_out=acc_dst)
                    chain = acc_dst

            nc.sync.dma_start(out=out.ap(), in_=out_sb[:, :])
    nc.compile()
    return nc


def _get_compiled():
    global _compiled
    if _compiled is None:
        _compiled = _build()
    return _compiled


def kernel(outputs: np.ndarray, targets: np.ndarray) -> np.ndarray:
    from concourse.bass_utils import run_bass_kernel_spmd

    outputs = np.asarray(outputs, dtype=np.float32)
    targets = np.asarray(targets, dtype=np.float32)
    assert outputs.shape == (NPTS, 3) and targets.shape == (NT, 3)

    nc = _get_compiled()
    tT = np.ascontiguousarray(targets.T)
    in_maps = []
    for c in range(N_CORES):
        shard = outputs[c * PP:(c + 1) * PP]
        in_maps.append({"outT": np.ascontiguousarray(shard.T), "tT": tT})

    res = run_bass_kernel_spmd(nc, in_maps, core_ids=list(range(N_CORES)))

    total = 0.0
    for c in range(N_CORES):
        o = res.results[c]["out"].astype(np.float64)
        total += o[:, 0:128].sum() + o[:, 128:144].sum()
    return np.float32(total / NPTS)



# revision 2
# speedup vs baseline: 13.1732x; 13.1732x over previous
"""ClosestPointLoss kernel for 8 trn2 NeuronCores — KD-pruned version.

mean_i min_j ||outputs_i - targets_j||^2 over outputs [131072,3], targets [16384,3].

Strategy: d^2(i,j) = |a_i|^2 + s_ij with s_ij = |t_j|^2 - 2 a_i.t_j; |a|^2 is
added on the host, so the device only needs min_j s_ij per point over a
CANDIDATE subset of targets that provably contains the nearest neighbor:

Host: KD-partition points into 1024 tiles of 128 (4 sub-boxes of 32 each).
For each tile, an upper bound UB on every member point's NN distance comes
from the 16 targets nearest the tile centroid; a target is a candidate if
its min distance to any sub-box is <= that sub-box's UB. This keeps ~190 of
16384 targets per tile (exact pruning, verified vs brute force).

Device: per point-tile slot, s_ij is a K=21 bf16 matmul (3 rows |t|^2
levels + 6 bf16-level cross products x 3 coords, exact to ~5e-6 abs).
Candidates are host-gathered into 256-col units; units run as 2 concurrent
matmuls in PE row bands (tile_position 0/64). A custom DVE op
(min(in0,in1) + running min-reduce) drains one unit from PSUM and one
ACT-copied unit from SBUF per op. The per-slot schedule (the "ladder") is
JIT-specialized to the data inside kernel(); one binary runs on all 8
cores (slot -> point-tile assignment is round-robin by descending work).

Host epilogue: sum the per-point mins, add sum|a|^2, divide by N.
"""
import sys

sys.path.insert(0, "/opt/trn_rl_repo")

import numpy as np
from contextlib import ExitStack

N_CORES = 8
NPTS = 131072
NT = 16384
P_LEAF = 128          # points per tile (PE partition dim)
SUB = 32              # points per sub-box (4 per tile)
NP_TILES = NPTS // P_LEAF   # 1024
NSLOT = NP_TILES // N_CORES # 128 slots per core
UNIT = 256            # candidate columns per matmul/drain unit
S_NEAR = 16           # targets per tile used for the UB bound
KROWS = 21            # matmul contraction rows
PAD_VAL = np.float32(1e30)

PAIRS = [("hi", "hi"), ("hi", "lo"), ("lo", "hi"),
         ("hi", "l2"), ("l2", "hi"), ("lo", "lo")]

_compiled = {}


# ---------------------------------------------------------------- host math
def _kd_order(pts, leaf):
    out = []

    def rec(ids):
        if len(ids) <= leaf:
            out.append(ids)
            return
        p = pts[ids]
        ax = int(np.argmax(p.max(0) - p.min(0)))
        k = len(ids) // 2
        part = np.argpartition(p[:, ax], k)
        rec(ids[part[:k]])
        rec(ids[part[k:]])

    rec(np.arange(pts.shape[0]))
    return np.concatenate(out)


def _levels(x):
    import ml_dtypes
    bf = ml_dtypes.bfloat16
    hi = x.astype(bf).astype(np.float32)
    r = x - hi
    lo = r.astype(bf).astype(np.float32)
    l2 = (r - lo).astype(bf).astype(np.float32)
    return {"hi": hi, "lo": lo, "l2": l2}


def _candidates(outputs, targets):
    """KD order + exact per-tile candidate target lists."""
    po = _kd_order(outputs, SUB)
    P = outputs[po].reshape(NP_TILES, P_LEAF, 3)
    Psub = outputs[po].reshape(NP_TILES, P_LEAF // SUB, SUB, 3)
    slo, shi = Psub.min(2), Psub.max(2)        # [NP,4,3]
    pc = 0.5 * (P.min(1) + P.max(1))

    UBs = np.empty((NP_TILES, P_LEAF // SUB), np.float64)
    blk = 128
    for i0 in range(0, NP_TILES, blk):
        i1 = min(NP_TILES, i0 + blk)
        d_c = ((pc[i0:i1, None, :] - targets[None, :, :]) ** 2).sum(-1)
        S = np.argpartition(d_c, S_NEAR, axis=1)[:, :S_NEAR]
        for j, i in enumerate(range(i0, i1)):
            dd = ((Psub[i][:, :, None, :] -
                   targets[S[j]][None, None, :, :]) ** 2).sum(-1)
            UBs[i] = dd.min(2).max(1)

    cand = []
    for i in range(NP_TILES):
        gap = np.maximum(0, np.maximum(targets[None, :, :] - shi[i][:, None, :],
                                       slo[i][:, None, :] - targets[None, :, :]))
        md2 = (gap ** 2).sum(-1)
        keep = (md2 <= UBs[i][:, None]).any(0)
        cand.append(np.nonzero(keep)[0])
    return po, cand


def _build_operands(outputs, targets, po, cand):
    """Schedule (ladder) + per-core W/R arrays + bookkeeping."""
    import ml_dtypes
    bf = ml_dtypes.bfloat16

    # 21-row level-split factors
    U = (targets.astype(np.float64) ** 2).sum(1).astype(np.float32)
    Ulv = _levels(U)                                   # [NT]
    Tlv = _levels((-2.0 * targets.astype(np.float64)).astype(np.float32))
    Rfull = np.zeros((KROWS, NT), np.float32)
    Rfull[0], Rfull[1], Rfull[2] = Ulv["hi"], Ulv["lo"], Ulv["l2"]
    for p, (_, rl) in enumerate(PAIRS):
        Rfull[3 + 3 * p:6 + 3 * p] = Tlv[rl].T
    Rfull = Rfull.astype(bf)

    A = outputs[po].astype(np.float32)                 # KD-ordered points
    Alv = _levels(A)
    Wfull = np.zeros((KROWS, NPTS), np.float32)
    Wfull[0:3] = 1.0
    for p, (wl, _) in enumerate(PAIRS):
        Wfull[3 + 3 * p:6 + 3 * p] = Alv[wl].T
    Wfull = Wfull.astype(bf)

    cnt = np.array([len(c) for c in cand])
    units = np.maximum(1, -(-cnt // UNIT))             # ceil
    order = np.argsort(-units, kind="stable")          # ptile ids, work desc
    ladder = units[order].reshape(NSLOT, N_CORES).max(1)  # [NSLOT]
    npairs = int(np.sum((ladder + 1) // 2))
    CW = npairs * UNIT

    W_dram = np.zeros((N_CORES, KROWS, NSLOT * P_LEAF), bf)
    R_dram = np.zeros((N_CORES, 2 * KROWS, CW), bf)
    # pad columns select nothing: row0 (|t|^2 hi level) = 1e30, rest 0
    R_dram[:, 0, :] = bf(PAD_VAL)
    R_dram[:, 1:KROWS, :] = bf(0.0)
    R_dram[:, KROWS, :] = bf(PAD_VAL)
    R_dram[:, KROWS + 1:, :] = bf(0.0)

    slot_ptile = np.empty((N_CORES, NSLOT), np.int64)
    for r in range(NSLOT):
        for c in range(N_CORES):
            pt = order[r * N_CORES + c]
            slot_ptile[c, r] = pt
            W_dram[c, :, r * P_LEAF:(r + 1) * P_LEAF] = \
                Wfull[:, pt * P_LEAF:(pt + 1) * P_LEAF]

    pair0 = np.zeros(NSLOT, np.int64)                  # first pair index of slot
    acc_p = 0
    for r in range(NSLOT):
        pair0[r] = acc_p
        acc_p += (int(ladder[r]) + 1) // 2
    assert acc_p == npairs

    for c in range(N_CORES):
        for r in range(NSLOT):
            pt = slot_ptile[c, r]
            cols = Rfull[:, cand[pt]]                  # [21, cnt]
            n = cols.shape[1]
            base = pair0[r] * UNIT
            for u in range(int(ladder[r])):
                lo_c, hi_c = u * UNIT, min(n, (u + 1) * UNIT)
                if lo_c >= n:
                    break
                band = (u % 2) * KROWS
                off = base + (u // 2) * UNIT
                R_dram[c, band:band + KROWS, off:off + hi_c - lo_c] = \
                    cols[:, lo_c:hi_c]
    return ladder, W_dram, R_dram, slot_ptile, CW


# ------------------------------------------------------------- device build
def _register_min_min_reduce():
    from concourse import dve_ops
    from concourse.dve_ops import DveOp, OPS, _SUB_OPCODE_FOR_NAME, _CUSTOM_DVE_ROW_BASE
    from concourse.dve_spec import Spec, Src0, Src1, C0, minn

    if "MIN_MIN_REDUCE" in _SUB_OPCODE_FOR_NAME:
        return dve_ops.MIN_MIN_REDUCE

    def _mmr_ref(in0, in1, c0, c1, c2):
        body = np.minimum(in0.astype(np.float32), in1.astype(np.float32))
        acc = np.minimum(np.asarray(c0, np.float32), body.min(axis=-1, keepdims=True))
        return body, acc

    op = DveOp(
        "MIN_MIN_REDUCE",
        Spec(body=minn(Src0, Src1), accum=minn, accum_init=C0, reference=_mmr_ref),
        subdim=False,
        uops_sha={},
    )
    from concourse.dve_ops import DveOpSpec, lower, has_src1

    for ver in ("v3", "v4"):
        spec = DveOpSpec(name=op.name, opcode=0, uops=lower(op.spec, ver=ver),
                         rd1_en=has_src1(op.spec))
        op.uops_sha[ver] = spec.sha(ver)
    OPS.append(op)
    _SUB_OPCODE_FOR_NAME[op.name] = _CUSTOM_DVE_ROW_BASE + len(OPS) - 1
    dve_ops.CUSTOM_DVE_SPECS[op.name] = op.spec
    dve_ops.MIN_MIN_REDUCE = op
    return op


def _build(ladder, CW):
    import concourse.bacc as bacc
    import concourse.tile as tile
    from concourse import mybir

    MMR = _register_min_min_reduce()
    f32 = mybir.dt.float32
    bf16 = mybir.dt.bfloat16

    ladder = [int(x) for x in ladder]
    npairs_slot = [(u + 1) // 2 for u in ladder]

    # chunk slots so each chunk's R cols fit a streamed SBUF tile
    CHUNK_PAIRS = 48
    chunks = []            # list of (slot_lo, slot_hi, pair_lo, pair_hi)
    s = 0
    pair_cursor = 0
    while s < NSLOT:
        e, pr = s, 0
        while e < NSLOT and (pr + npairs_slot[e] <= CHUNK_PAIRS or pr == 0):
            pr += npairs_slot[e]
            e += 1
        chunks.append((s, e, pair_cursor, pair_cursor + pr))
        pair_cursor += pr
        s = e
    assert pair_cursor * UNIT == CW

    nc = bacc.Bacc("TRN2", target_bir_lowering=False, debug=False)
    Wd = nc.dram_tensor("Wd", [KROWS, NSLOT * P_LEAF], bf16, kind="ExternalInput")
    Rd = nc.dram_tensor("Rd", [2 * KROWS, CW], bf16, kind="ExternalInput")
    out = nc.dram_tensor("out", [128, NSLOT], f32, kind="ExternalOutput")

    with tile.TileContext(nc) as tc:
        with ExitStack() as ctx:
            singles = ctx.enter_context(tc.tile_pool(name="singles", bufs=1))
            Wsb = singles.tile([128, NSLOT * P_LEAF], bf16)
            out_sb = singles.tile([128, NSLOT], f32)
            inf_t = singles.tile([128, UNIT], f32)
            dump = singles.tile([128, 1], f32)

            nc.vector.memset(inf_t[:, :], 3.0e38)
            nc.sync.dma_start(out=Wsb[0:KROWS, :], in_=Wd.ap())
            nc.sync.dma_start(out=Wsb[64:64 + KROWS, :], in_=Wd.ap())

            r_pool = ctx.enter_context(tc.tile_pool(name="rp", bufs=2))
            pa_pool = ctx.enter_context(tc.tile_pool(name="pa", bufs=3, space="PSUM"))
            pb_pool = ctx.enter_context(tc.tile_pool(name="pb", bufs=3, space="PSUM"))
            cp_pool = ctx.enter_context(tc.tile_pool(name="cp", bufs=3))
            acc_pool = ctx.enter_context(tc.tile_pool(name="accp", bufs=4))

            for (s_lo, s_hi, p_lo, p_hi) in chunks:
                ccols = (p_hi - p_lo) * UNIT
                rt = r_pool.tile([128, ccols], bf16, name="rt", tag="rt")
                c0 = p_lo * UNIT
                nc.sync.dma_start(out=rt[0:KROWS, :],
                                  in_=Rd.ap()[0:KROWS, c0:c0 + ccols])
                nc.sync.dma_start(out=rt[64:64 + KROWS, :],
                                  in_=Rd.ap()[KROWS:2 * KROWS, c0:c0 + ccols])
                pair_off = 0
                for r in range(s_lo, s_hi):
                    ws = slice(r * P_LEAF, (r + 1) * P_LEAF)
                    chain = 3.0e38
                    nu = ladder[r]
                    for k in range(0, nu, 2):
                        col = (pair_off + k // 2) * UNIT
                        pa = pa_pool.tile([128, 512], f32, name="pat", tag="pat")
                        nc.tensor.matmul(pa[:, 0:UNIT], Wsb[0:KROWS, ws],
                                         rt[0:KROWS, col:col + UNIT],
                                         start=True, stop=True, tile_position=(0, 0))
                        last = k + 2 >= nu
                        acc_dst = out_sb[:, r:r + 1] if last else \
                            acc_pool.tile([128, 1], f32, name="acct", tag="acct")
                        if k + 1 < nu:
                            pb = pb_pool.tile([128, 512], f32, name="pbt", tag="pbt")
                            nc.tensor.matmul(pb[:, 0:UNIT], Wsb[64:64 + KROWS, ws],
                                             rt[64:64 + KROWS, col:col + UNIT],
                                             start=True, stop=True, tile_position=(64, 0))
                            cpt = cp_pool.tile([128, UNIT], f32, name="cpt", tag="cpt")
                            nc.scalar.copy(cpt[:, :], pb[:, 0:UNIT])
                            in1 = cpt[:, :]
                        else:
                            in1 = inf_t[:, :]
                        nc.vector._custom_dve(MMR, out=dump.broadcast_to((128, UNIT)),
                                              in0=pa[:, 0:UNIT], in1=in1, s0=chain,
                                              accum_out=acc_dst)
                        chain = acc_dst
                    pair_off += npairs_slot[r]

            nc.sync.dma_start(out=out.ap(), in_=out_sb[:, :])
    nc.compile()
    return nc


def _get_compiled(ladder, CW):
    key = (tuple(int(x) for x in ladder), int(CW))
    if key not in _compiled:
        _compiled[key] = _build(ladder, CW)
    return _compiled[key]


# ------------------------------------------------------------------- kernel
def kernel(outputs: np.ndarray, targets: np.ndarray) -> np.ndarray:
    from concourse.bass_utils import run_bass_kernel_spmd

    outputs = np.asarray(outputs, dtype=np.float32)
    targets = np.asarray(targets, dtype=np.float32)
    assert outputs.shape == (NPTS, 3) and targets.shape == (NT, 3)

    po, cand = _candidates(outputs, targets)
    ladder, W_dram, R_dram, slot_ptile, CW = _build_operands(
        outputs, targets, po, cand)

    nc = _get_compiled(ladder, CW)
    in_maps = [{"Wd": np.ascontiguousarray(W_dram[c]),
                "Rd": np.ascontiguousarray(R_dram[c])}
               for c in range(N_CORES)]
    res = run_bass_kernel_spmd(nc, in_maps, core_ids=list(range(N_CORES)))

    total = float((outputs.astype(np.float64) ** 2).sum())
    for c in range(N_CORES):
        total += res.results[c]["out"].astype(np.float64).sum()
    return np.float32(total / NPTS)


# revision 9
# speedup vs baseline: 36.7264x; 2.7880x over previous
"""ClosestPointLoss kernel for 8 trn2 NeuronCores — KD-pruned, scan-drained.

mean_i min_j ||outputs_i - targets_j||^2 over outputs [131072,3], targets [16384,3].

Host: KD-partition points into 1024 tiles ("slots") of 128; exact pruning
keeps ~190 of 16384 candidate targets per tile (upper bound from the 16
targets nearest each tile centroid; a target survives if its distance lower
bound to any 32-point sub-box beats that sub-box's bound). Verified exact
vs brute force.

Device: d^2(i,j) is a K=25 bf16 level-split matmul (rows: 3x |t|^2 levels,
18 cross-product rows, 3x |a|^2 levels, 1 offset row) — abs err ~5e-6.
Candidates are gathered into 128-col-padded slot blocks, packed into
2048-col PSUM groups (matmuls clipped at 512-col bank edges, alternating
two PE row bands). The whole group drains with ONE custom DVE op: an
inclusive prefix-min scan whose output AP is stride-0 within 128-element
pages, so each out column holds the scan value at that page end. A
per-slot additive offset (strictly increasing down the group, baked into
the offset matmul row) makes every later slot's values smaller than every
earlier slot's, so the scan value at a slot's last page IS that slot's
row-min; the host adds the offset back. This needs ~17 DVE ops per core
instead of one-per-slot (128+), sidestepping the ~350ns/op fixed cost.

Host epilogue: min over group-pieces per slot, sum, divide by N.
"""
import sys

sys.path.insert(0, "/opt/trn_rl_repo")

import numpy as np
from contextlib import ExitStack

N_CORES = 8
NPTS = 131072
NT = 16384
P_LEAF = 128          # points per slot (PE partition dim)
SUB = 16              # points per sub-box
NP_TILES = NPTS // P_LEAF   # 1024
NSLOT = NP_TILES // N_CORES # 128 slots per core
S_NEAR = 32           # targets per tile used for the UB bound
KROWS = 25            # matmul contraction rows (incl |a|^2 + offset rows)
GROUP = 2048          # cols per PSUM group (4 banks)
PAGE = 64             # scan output sampling page
CHUNK_GROUPS = 4      # R-streaming chunk size
PAD_VAL = np.float32(1e30)

PAIRS = [("hi", "hi"), ("hi", "lo"), ("lo", "hi"),
         ("hi", "l2"), ("l2", "hi"), ("lo", "lo")]

_compiled = {}


# ---------------------------------------------------------------- host math
def _kd_order(pts, leaf):
    out = []

    def rec(ids):
        if len(ids) <= leaf:
            out.append(ids)
            return
        p = pts[ids]
        ax = int(np.argmax(p.max(0) - p.min(0)))
        k = len(ids) // 2
        part = np.argpartition(p[:, ax], k)
        rec(ids[part[:k]])
        rec(ids[part[k:]])

    rec(np.arange(pts.shape[0]))
    return np.concatenate(out)


def _levels(x):
    import ml_dtypes
    bf = ml_dtypes.bfloat16
    hi = x.astype(bf).astype(np.float32)
    r = x - hi
    lo = r.astype(bf).astype(np.float32)
    l2 = (r - lo).astype(bf).astype(np.float32)
    return {"hi": hi, "lo": lo, "l2": l2}


def _candidates(outputs, targets):
    """KD order + exact per-tile candidate lists + per-tile max-dist bound D."""
    po = _kd_order(outputs, SUB)
    P = outputs[po].reshape(NP_TILES, P_LEAF, 3)
    Psub = outputs[po].reshape(NP_TILES, P_LEAF // SUB, SUB, 3)
    slo, shi = Psub.min(2), Psub.max(2)
    plo, phi = P.min(1), P.max(1)
    pc = 0.5 * (plo + phi)

    UBs = np.empty((NP_TILES, P_LEAF // SUB), np.float64)
    blk = 64
    for i0 in range(0, NP_TILES, blk):
        i1 = min(NP_TILES, i0 + blk)
        d_c = ((pc[i0:i1, None, :] - targets[None, :, :]) ** 2).sum(-1)
        S = np.argpartition(d_c, S_NEAR, axis=1)[:, :S_NEAR]
        ts = targets[S]                                   # [B,S,3]
        diff = Psub[i0:i1, :, :, None, :] - ts[:, None, None, :, :]
        dd = (diff ** 2).sum(-1)                          # [B,ns,SUB,S]
        UBs[i0:i1] = dd.min(3).max(2)

    cand, Dmax = [], np.empty(NP_TILES, np.float64)
    for i in range(NP_TILES):
        gap = np.maximum(0, np.maximum(targets[None, :, :] - shi[i][:, None, :],
                                       slo[i][:, None, :] - targets[None, :, :]))
        md2 = (gap ** 2).sum(-1)
        keep = (md2 <= UBs[i][:, None]).any(0)
        idx = np.nonzero(keep)[0]
        cand.append(idx)
        far = np.maximum(np.abs(targets[idx] - plo[i]),
                         np.abs(targets[idx] - phi[i]))
        Dmax[i] = (far ** 2).sum(-1).max()
    return po, cand, Dmax


def _schedule(cand):
    """Shared (core-independent) static schedule from the padded ladder."""
    cnt = np.array([len(c) for c in cand])
    cols = np.maximum(PAGE, -(-cnt // PAGE) * PAGE)      # 128-col padded
    order = np.argsort(-cols, kind="stable")             # ptile ids by work desc
    ladder = cols[order].reshape(NSLOT, N_CORES).max(1)  # [NSLOT] shared

    groups = []        # each: {'L', 'segs': [(psum_off, cols, r, slot_off, band)]}
    pieces = []        # (r, group_idx, end_pos, piece_cols) in stream order
    cur = {"L": 0, "segs": []}

    def close():
        nonlocal cur
        if cur["L"]:
            groups.append(cur)
            cur = {"L": 0, "segs": []}

    for r in range(NSLOT):
        rem = int(ladder[r])
        slot_off = 0
        while rem:
            if cur["L"] >= GROUP:
                close()
            take = min(rem, GROUP - cur["L"])
            # emit segments clipped at 512-col bank edges
            p = cur["L"]
            left = take
            so = slot_off
            while left:
                seg = min(left, 512 - (p % 512))
                cur["segs"].append((p, seg, r, so))
                p += seg
                so += seg
                left -= seg
            pieces.append((r, len(groups), cur["L"] + take, take, slot_off))
            cur["L"] += take
            slot_off += take
            rem -= take
    close()

    pages0, np_ = [], 0
    for g in groups:
        pages0.append(np_)
        np_ += g["L"] // PAGE
    npages = np_

    # chunks of consecutive groups (first chunk = 1 group for a fast start)
    chunks = []
    bounds = [0, 1, 3]
    while bounds[-1] < len(groups):
        bounds.append(min(len(groups), bounds[-1] + CHUNK_GROUPS))
    bounds = sorted(set(min(b, len(groups)) for b in bounds))
    for c0, c1 in zip(bounds[:-1], bounds[1:]):
        gs = list(range(c0, c1))
        bcols = 0
        seg_rt = {}
        w_slots = set()
        for gi in gs:
            for (off, seg, r, so) in groups[gi]["segs"]:
                seg_rt[(gi, off)] = bcols
                bcols += seg
                w_slots.add(r)
        chunks.append({"groups": gs, "bcols": bcols, "rt": seg_rt,
                       "w_slots": (min(w_slots), max(w_slots))})
    CWB = sum(ch["bcols"] for ch in chunks)

    # per-slot sample list: (group, out_page_col)
    samples = {r: [] for r in range(NSLOT)}
    for k, (r, gi, end, pcols, soff) in enumerate(pieces):
        samples[r].append((gi, pages0[gi] + end // PAGE - 1, k))

    return dict(ladder=ladder, order=order, groups=groups, pieces=pieces,
                pages0=pages0, npages=npages, chunks=chunks, CWB=CWB,
                samples=samples)


def _build_operands(outputs, targets, po, cand, Dmax, sched):
    """Per-core W [50,NSLOT*128] / R [50,CWB] bf16 arrays + per-piece offsets."""
    import ml_dtypes
    bf = ml_dtypes.bfloat16

    U = (targets.astype(np.float64) ** 2).sum(1).astype(np.float32)
    Ulv = _levels(U)
    Tlv = _levels((-2.0 * targets.astype(np.float64)).astype(np.float32))
    Rfull = np.zeros((KROWS, NT), np.float32)
    Rfull[0], Rfull[1], Rfull[2] = Ulv["hi"], Ulv["lo"], Ulv["l2"]
    for p, (_, rl) in enumerate(PAIRS):
        Rfull[3 + 3 * p:6 + 3 * p] = Tlv[rl].T
    Rfull[21:24] = 1.0
    # row 24 (offset) set per-column during gather
    Rfull = Rfull.astype(bf).astype(np.float32)

    A = outputs[po].astype(np.float32)
    Alv = _levels(A)
    a2 = (outputs[po].astype(np.float64) ** 2).sum(1).astype(np.float32)
    a2lv = _levels(a2)
    Wfull = np.zeros((KROWS, NPTS), np.float32)
    Wfull[0:3] = 1.0
    for p, (wl, _) in enumerate(PAIRS):
        Wfull[3 + 3 * p:6 + 3 * p] = Alv[wl].T
    Wfull[21], Wfull[22], Wfull[23] = a2lv["hi"], a2lv["lo"], a2lv["l2"]
    Wfull[24] = 1.0
    Wfull = Wfull.astype(bf)

    order, ladder = sched["order"], sched["ladder"]
    groups, pieces, chunks = sched["groups"], sched["pieces"], sched["chunks"]

    W_dram = np.zeros((N_CORES, KROWS, NSLOT * P_LEAF), bf)
    R_dram = np.zeros((N_CORES, KROWS, sched["CWB"]), bf)
    offs = np.zeros((N_CORES, len(pieces)), np.float64)

    slot_ptile = np.empty((N_CORES, NSLOT), np.int64)
    for r in range(NSLOT):
        for c in range(N_CORES):
            pt = order[r * N_CORES + c]
            slot_ptile[c, r] = pt
            W_dram[c, :, r * P_LEAF:(r + 1) * P_LEAF] = \
                Wfull[:, pt * P_LEAF:(pt + 1) * P_LEAF]

    # per-core gathered candidate columns per slot (padded by replication)
    for c in range(N_CORES):
        slot_cols = {}
        for r in range(NSLOT):
            pt = slot_ptile[c, r]
            idx = cand[pt]
            n, padto = len(idx), int(ladder[r])
            idx = np.concatenate([idx, np.full(padto - n, idx[0])]) if n < padto else idx
            slot_cols[r] = Rfull[:, idx]          # [25, ladder[r]] f32

        # offsets per piece (reset each group, increasing within)
        piece_off = {}
        for gi in range(len(groups)):
            o = 0.0
            first = True
            for k, (r, g2, end, pcols, soff) in enumerate(pieces):
                if g2 != gi:
                    continue
                if not first:
                    o = o + np.ceil(Dmax[slot_ptile[c, r]]) + 1.0
                first = False
                piece_off[k] = o
                offs[c, k] = o
        assert max(piece_off.values()) <= 500, "offset overflow"

        # fill R: walk chunks/segments
        cw0 = 0
        for ch in chunks:
            for gi in ch["groups"]:
                for (off, seg, r, so) in groups[gi]["segs"]:
                    ok = [k for k, pc_ in enumerate(pieces)
                          if pc_[0] == r and pc_[1] == gi]
                    o = piece_off[ok[0]]
                    colblk = slot_cols[r][:, so:so + seg].copy()
                    colblk[24] = -o
                    rt = ch["rt"][(gi, off)]
                    R_dram[c, :, cw0 + rt:cw0 + rt + seg] = \
                        colblk.astype(R_dram.dtype)
            cw0 += ch["bcols"]
    return W_dram, R_dram, offs, slot_ptile


# ------------------------------------------------------------- device build
def _register_min_scan():
    from concourse import dve_ops
    from concourse.dve_ops import DveOp, OPS, _SUB_OPCODE_FOR_NAME, _CUSTOM_DVE_ROW_BASE
    from concourse.dve_spec import Spec, Src0, C0, Scan, minn, Zero

    if "MIN_SCAN_V1" in _SUB_OPCODE_FOR_NAME:
        return dve_ops.MIN_SCAN_V1

    MINOP = minn(Zero, Zero).op

    def _ref(in0, in1, c0, c1, c2):
        flat = in0.reshape(in0.shape[0], -1).astype(np.float32)
        sc = np.minimum.accumulate(flat, axis=-1)
        sc = np.minimum(sc, np.asarray(c0, np.float32).reshape(-1, 1))
        return sc.reshape(in0.shape)

    op = DveOp(
        "MIN_SCAN_V1",
        Spec(body=Scan(MINOP, Src0, init=C0), reference=_ref),
        subdim=False,
        uops_sha={},
    )
    from concourse.dve_ops import DveOpSpec, lower, has_src1

    for ver in ("v3", "v4"):
        spec = DveOpSpec(name=op.name, opcode=0, uops=lower(op.spec, ver=ver),
                         rd1_en=has_src1(op.spec))
        op.uops_sha[ver] = spec.sha(ver)
    OPS.append(op)
    _SUB_OPCODE_FOR_NAME[op.name] = _CUSTOM_DVE_ROW_BASE + len(OPS) - 1
    dve_ops.CUSTOM_DVE_SPECS[op.name] = op.spec
    dve_ops.MIN_SCAN_V1 = op
    return op


def _build(sched):
    import concourse.bacc as bacc
    import concourse.tile as tile
    from concourse import mybir

    MSC = _register_min_scan()
    f32 = mybir.dt.float32
    bf16 = mybir.dt.bfloat16

    groups, chunks = sched["groups"], sched["chunks"]
    npages, CWB = sched["npages"], sched["CWB"]

    nc = bacc.Bacc("TRN2", target_bir_lowering=False, debug=False)
    Wd = nc.dram_tensor("Wd", [KROWS, NSLOT * P_LEAF], bf16, kind="ExternalInput")
    Rd = nc.dram_tensor("Rd", [KROWS, CWB], bf16, kind="ExternalInput")
    out = nc.dram_tensor("out", [128, npages], f32, kind="ExternalOutput")

    with tile.TileContext(nc) as tc:
        with ExitStack() as ctx:
            singles = ctx.enter_context(tc.tile_pool(name="singles", bufs=1))
            Wsb = singles.tile([128, NSLOT * P_LEAF], bf16)
            out_sb = singles.tile([128, npages], f32)

            r_pool = ctx.enter_context(tc.tile_pool(name="rp", bufs=2))
            g_pool = ctx.enter_context(tc.tile_pool(name="gp", bufs=2, space="PSUM"))

            w_done = -1
            cw0 = 0
            for ch in chunks:
                w_lo, w_hi = ch["w_slots"]
                w_lo = max(w_lo, w_done + 1)
                if w_hi >= w_lo:
                    cs = slice(w_lo * P_LEAF, (w_hi + 1) * P_LEAF)
                    nc.sync.dma_start(out=Wsb[0:KROWS, cs], in_=Wd.ap()[:, cs])
                    w_done = w_hi
                bc = ch["bcols"]
                rt = r_pool.tile([128, bc], bf16, name="rt", tag="rt")
                nc.sync.dma_start(out=rt[0:KROWS, :],
                                  in_=Rd.ap()[:, cw0:cw0 + bc])

                for gi in ch["groups"]:
                    g = groups[gi]
                    L = g["L"]
                    gt = g_pool.tile([128, GROUP], f32, name="gt", tag="gt")
                    for (off, seg, r, so) in g["segs"]:
                        rto = ch["rt"][(gi, off)]
                        nc.tensor.matmul(
                            gt[:, off:off + seg],
                            Wsb[0:KROWS, r * P_LEAF:(r + 1) * P_LEAF],
                            rt[0:KROWS, rto:rto + seg],
                            start=True, stop=True, tile_position=(0, 0))
                    P = L // PAGE
                    p0 = sched["pages0"][gi]
                    in3 = gt[:, 0:L].rearrange("p (s o) -> p s o", o=PAGE)
                    out3 = out_sb[:, p0:p0 + P].rearrange(
                        "p (s o) -> p s o", o=1).broadcast_to((128, P, PAGE))
                    nc.vector._custom_dve(MSC, out=out3, in0=in3, s0=3.0e38)
                cw0 += ch["bcols"]

            nc.sync.dma_start(out=out.ap(), in_=out_sb[:, :])
    nc.compile()
    return nc


def _sched_key(sched):
    return (tuple(int(x) for x in sched["ladder"]), sched["CWB"], sched["npages"])


def _get_compiled(sched):
    key = _sched_key(sched)
    if key not in _compiled:
        _compiled[key] = _build(sched)
    return _compiled[key]


# ------------------------------------------------------------------- kernel
def kernel(outputs: np.ndarray, targets: np.ndarray) -> np.ndarray:
    from concourse.bass_utils import run_bass_kernel_spmd

    outputs = np.asarray(outputs, dtype=np.float32)
    targets = np.asarray(targets, dtype=np.float32)
    assert outputs.shape == (NPTS, 3) and targets.shape == (NT, 3)

    po, cand, Dmax = _candidates(outputs, targets)
    sched = _schedule(cand)
    W_dram, R_dram, offs, slot_ptile = _build_operands(
        outputs, targets, po, cand, Dmax, sched)

    nc = _get_compiled(sched)
    in_maps = [{"Wd": np.ascontiguousarray(W_dram[c]),
                "Rd": np.ascontiguousarray(R_dram[c])}
               for c in range(N_CORES)]
    res = run_bass_kernel_spmd(nc, in_maps, core_ids=list(range(N_CORES)))

    total = 0.0
    for c in range(N_CORES):
        o = res.results[c]["out"].astype(np.float64)
        for r in range(NSLOT):
            best = None
            for (gi, col, k) in sched["samples"][r]:
                v = o[:, col] + offs[c, k]
                best = v if best is None else np.minimum(best, v)
            total += best.sum()
    return np.float32(total / NPTS)
